# revision 25
# baseline (speedup 1.0000x reference)
"""BinaryTreeLSTM on 8 Trainium2 NeuronCores — feature-major fp16 pipeline.

Data-parallel over the leaf batch: core d owns leaves [1024d, 1024d+1024)
as 32 independent 32-leaf subtrees, folded through the leaf LSTM plus 5
merge levels (B = 512, 256, 128, 64, 32). The 256 subtree roots
(32 per core) are gathered on host, which folds the remaining 8 levels
(255 nodes, ~3% of FLOPs).

Everything stays feature-major on every level:
- matmul: stationary = weight chunk [128, 128 gate cols] (fp16 -> FWL),
  moving = child h tile [128, B]. fp16 avoids the fp32r 4x penalty below
  256 moving columns, so the small tail levels stay cheap.
- h is stored [128, half, parity, node]: the even/odd child split is one
  fused DVE write, and next-level matmuls slice it contiguously.
- leaf bias rides in the K-padding (embs row 300 = 1, Wx row 300 = bx),
  so leaf activations need no bias and cover both halves in one instr.
- c stays fp32 (root values reach ~1.5e3); gates stay fp32, h is fp16.
"""

import numpy as np

IN_DIM = 300
KPAD = 384
MEM_DIM = 256
N_LEAVES = 8192
N_CORES = 8
LPC = N_LEAVES // N_CORES   # 1024 leaves per core
N_SUB = 32                  # subtrees per core -> 32 roots per core

# (gate, half) -> px m-chunk ([u,i,lf,rf,o] x 2; lf/rf share fx)
_PXCOL = [0, 1, 2, 3, 4, 5, 4, 5, 6, 7]

_CACHE = {}


def _build():
    import concourse.bacc as bacc
    import concourse.mybir as mybir
    import concourse.tile as tile

    f32 = mybir.dt.float32
    fp16 = mybir.dt.float16
    AF = mybir.ActivationFunctionType

    nc = bacc.Bacc("TRN2", target_bir_lowering=False, debug=False,
                   num_devices=N_CORES)

    embsT = nc.dram_tensor("embsT", [KPAD, LPC], fp16, kind="ExternalInput").ap()
    WxT = nc.dram_tensor("WxT", [KPAD, 768], fp16, kind="ExternalInput").ap()
    WlT = nc.dram_tensor("WlT", [MEM_DIM, 1280], fp16, kind="ExternalInput").ap()
    WrT = nc.dram_tensor("WrT", [MEM_DIM, 1280], fp16, kind="ExternalInput").ap()
    pxf = nc.dram_tensor("pxf", [128, 10], f32, kind="ExternalInput").ap()
    pxrow = nc.dram_tensor("pxrow", [1, 1280], fp16, kind="ExternalInput").ap()
    out = nc.dram_tensor("out", [128, 4 * N_SUB], f32,
                         kind="ExternalOutput").ap()

    with tile.TileContext(nc) as tc:
        with (
            tc.tile_pool(name="const", bufs=1) as const,
            tc.tile_pool(name="state", bufs=1) as state,
            tc.tile_pool(name="gates", bufs=2) as gates,
            tc.tile_pool(name="psum", bufs=1, space="PSUM") as psum,
        ):
            v2 = lambda t: t.rearrange("p (c n) -> p c n", c=2)

            # ---- PE warm-up on a zeroed scratch (no DMA dependency) ----
            warm = const.tile([128, 512], fp16)
            nc.vector.memset(warm[:, :], 0.0)
            ones = const.tile([1, 128], fp16)
            nc.vector.memset(ones[:, :], 1.0)
            wps = psum.tile([128, 512], f32, tag="mg3", name="warmps")
            for wi in range(6):
                nc.tensor.matmul(wps[:, :], warm[:, 0:128], warm[:, :],
                                 start=(wi == 0), stop=(wi == 5))

            # ---- all input DMAs on one ordered Sync queue: leaf deps
            # first, Wl/Wr before L1 needs them, pxf last ----
            Wx_sb = const.tile([128, 3 * 768], fp16)
            embs_sb = const.tile([128, 3 * LPC], fp16)
            e3 = embs_sb.rearrange("p (k n) -> p k n", k=3)
            ed = embsT.rearrange("(k p) n -> p k n", p=128)
            Wl_sb = const.tile([128, 2 * 1280], fp16)
            Wr_sb = const.tile([128, 2 * 1280], fp16)
            px_fm = const.tile([128, 10], f32)
            for k in range(3):
                nc.sync.dma_start(Wx_sb[:, k * 768:(k + 1) * 768],
                                  WxT[128 * k:128 * (k + 1), :])
                nc.sync.dma_start(e3[:, k, 0:512], ed[:, k, 0:512])
            for k in range(3):
                nc.sync.dma_start(e3[:, k, 512:1024], ed[:, k, 512:1024])
            nc.sync.dma_start(
                Wl_sb.rearrange("p (k f) -> p k f", k=2),
                WlT.rearrange("(k p) f -> p k f", p=128))
            nc.sync.dma_start(
                Wr_sb.rearrange("p (k f) -> p k f", k=2),
                WrT.rearrange("(k p) f -> p k f", p=128))
            nc.sync.dma_start(px_fm[:, :], pxf[:, :])
            pxr_sb = const.tile([1, 1280], fp16)
            nc.sync.dma_start(pxr_sb[:, :], pxrow[:, :])

            # h tiles: [128, (half, parity, k)] -> col = half*B + par*B/2 + k
            def hview(t, B):
                return t.rearrange("p (c q n) -> p c q n", c=2, q=2)

            # ---- leaf phase: B=1024, 2 chunks of 512; bias in K-pad ----
            c0 = state.tile([128, 2 * LPC], f32, tag="c0")
            h0 = state.tile([128, 2 * LPC], fp16, tag="h0")
            c0_3, h0_4 = v2(c0), hview(h0, LPC)
            for sg in range(2):
                gt = {}
                for gi, gname in enumerate(("u", "i", "o")):
                    gt[gname] = psum.tile([128, 2 * 512], f32, tag=f"mg{gi}",
                                          name=f"x{gname}{sg}")
                # k-outer so the first matmuls gate only on the k0 DMAs
                for ki in range(3):
                    for gi, gname in enumerate(("u", "i", "o")):
                        for half in range(2):
                            m = gi * 2 + half
                            dst = gt[gname][:, half * 512:(half + 1) * 512]
                            nc.tensor.matmul(
                                dst,
                                Wx_sb[:, ki * 768 + m * 128:
                                      ki * 768 + (m + 1) * 128],
                                embs_sb[:, ki * LPC + sg * 512:
                                        ki * LPC + (sg + 1) * 512],
                                start=(ki == 0), stop=(ki == 2))
                ut = gates.tile([128, 2 * 512], fp16, tag="mu", name=f"u{sg}")
                it = gates.tile([128, 2 * 512], fp16, tag="mi", name=f"i{sg}")
                ot = gates.tile([128, 2 * 512], fp16, tag="mo", name=f"o{sg}")
                tht = gates.tile([128, 2 * 512], fp16, tag="mth", name=f"th{sg}")
                for gname, dst, fn in (("i", it, AF.Sigmoid), ("u", ut, AF.Tanh),
                                       ("o", ot, AF.Sigmoid)):
                    nc.scalar.activation(dst[:, :], gt[gname][:, :], fn)
                cs = c0_3[:, :, sg * 512:(sg + 1) * 512]
                u3, i3, o3, th3 = v2(ut), v2(it), v2(ot), v2(tht)
                nc.vector.tensor_mul(cs, i3, u3)
                nc.scalar.activation(th3, cs, AF.Tanh)
                # even/odd split writes; parity 0 first (consumed first)
                hd = h0_4[:, :, :, sg * 256:(sg + 1) * 256]
                o4 = ot.rearrange("p (c n q) -> p c q n", c=2, q=2)
                th4 = tht.rearrange("p (c n q) -> p c q n", c=2, q=2)
                nc.vector.tensor_mul(hd[:, :, 0, :], o4[:, :, 0, :],
                                     th4[:, :, 0, :])
                nc.vector.tensor_mul(hd[:, :, 1, :], o4[:, :, 1, :],
                                     th4[:, :, 1, :])

            # ---- merge levels, all feature-major ----
            GATE_FNS = (AF.Tanh, AF.Sigmoid, AF.Sigmoid, AF.Sigmoid,
                        AF.Sigmoid)

            def fm_level(cp, hp, B, lvl):
                """children: cp [128,2,2B] f32, hp flat [128, 2*2B] fp16
                in (half, parity, k) layout. Returns (c, h) tiles."""
                last = B == N_SUB
                Bp = 2 * B  # children per half-row pair
                cn = state.tile([128, 2 * B], f32, tag=f"c{lvl}")
                if last:
                    hn = state.tile([128, 2 * B], f32, tag=f"h{lvl}")
                else:
                    hn = state.tile([128, 2 * B], fp16, tag=f"h{lvl}")
                # 2 chunks when B >= 256 so the next level can start on
                # chunk 0's h while chunk 1 is still in flight
                GC = min(256, B // 2) if B >= 256 else B
                ORDER = ("lf", "rf", "u", "i", "o")
                GIDX = {"u": 0, "i": 1, "lf": 2, "rf": 3, "o": 4}
                FNS = {"u": AF.Tanh, "i": AF.Sigmoid, "lf": AF.Sigmoid,
                       "rf": AF.Sigmoid, "o": AF.Sigmoid}
                for g0 in range(0, B, GC):
                    G = GC
                    sfx = f"{lvl}_{g0}"
                    small = G <= 128
                    gt = {}
                    for slot, gname in enumerate(ORDER):
                        # slots rotated so the first gates (lf, rf) use
                        # tags the leaf never touches: no leaf->L1 psum WAR
                        t = psum.tile([128, 2 * G], f32,
                                      tag=f"mg{(slot + 3) % 5}",
                                      name=f"g{sfx}_{gname}")
                        for half in range(2):
                            m = GIDX[gname] * 2 + half
                            dst = t[:, half * G:(half + 1) * G]
                            if small:  # px pre-add: K=1 ones matmul
                                nc.tensor.matmul(
                                    dst, pxr_sb[0:1, m * 128:(m + 1) * 128],
                                    ones[:, 0:G], start=True, stop=False)
                            for ki in range(4):
                                W = Wl_sb if ki < 2 else Wr_sb
                                kc = ki % 2
                                # parity 0 (even) for Wl, 1 (odd) for Wr
                                hsrc = hp[:, kc * Bp + (ki // 2) * B +
                                          g0:kc * Bp + (ki // 2) * B + g0 + G]
                                nc.tensor.matmul(
                                    dst,
                                    W[:, kc * 1280 + m * 128:
                                      kc * 1280 + (m + 1) * 128],
                                    hsrc,
                                    start=(not small and ki == 0),
                                    stop=(ki == 3))
                        gt[gname] = t
                    ga = {}
                    for gname in ORDER:
                        a = gates.tile([128, 2 * G], fp16, tag=f"m{gname}",
                                       name=f"{gname}{sfx}")
                        if small:
                            nc.scalar.activation(a[:, :], gt[gname][:, :],
                                                 FNS[gname])
                        else:
                            for half in range(2):
                                pc = GIDX[gname] * 2 + half
                                nc.scalar.activation(
                                    a[:, half * G:(half + 1) * G],
                                    gt[gname][:, half * G:(half + 1) * G],
                                    FNS[gname], bias=px_fm[:, pc:pc + 1])
                        ga[gname] = a
                    lc = cp[:, :, 2 * g0:2 * (g0 + G):2]
                    rc = cp[:, :, 2 * g0 + 1:2 * (g0 + G):2]
                    x1 = gates.tile([128, 2 * G], fp16, tag="x1", name=f"x1{sfx}")
                    x2 = gates.tile([128, 2 * G], f32, tag="x2", name=f"x2{sfx}")
                    x3 = gates.tile([128, 2 * G], f32, tag="x3", name=f"x3{sfx}")
                    s1 = gates.tile([128, 2 * G], f32, tag="s1", name=f"s1{sfx}")
                    tht = gates.tile([128, 2 * G], fp16, tag="mth",
                                     name=f"th{sfx}")
                    x13, x23, x33, s13, th3 = v2(x1), v2(x2), v2(x3), v2(s1), v2(tht)
                    cs = v2(cn)[:, :, g0:g0 + G]
                    # c-chain: f-terms first (available early), i*u last
                    nc.vector.tensor_mul(x23, v2(ga["lf"]), lc)
                    nc.vector.tensor_mul(x33, v2(ga["rf"]), rc)
                    nc.vector.tensor_add(s13, x23, x33)
                    nc.vector.tensor_mul(x13, v2(ga["u"]), v2(ga["i"]))
                    nc.vector.tensor_add(cs, s13, x13)
                    nc.scalar.activation(th3, cs, AF.Tanh)
                    if last:
                        nc.vector.tensor_mul(v2(hn)[:, :, g0:g0 + G],
                                             v2(ga["o"]), th3)
                    else:
                        hd = hview(hn, B)[:, :, :, g0 // 2:(g0 + G) // 2]
                        o4 = ga["o"].rearrange("p (c n q) -> p c q n", c=2, q=2)
                        th4 = tht.rearrange("p (c n q) -> p c q n", c=2, q=2)
                        nc.vector.tensor_mul(hd[:, :, 0, :], o4[:, :, 0, :],
                                             th4[:, :, 0, :])
                        nc.vector.tensor_mul(hd[:, :, 1, :], o4[:, :, 1, :],
                                             th4[:, :, 1, :])
                return cn, hn

            cp, hp = c0, h0
            B, lvl = 512, 1
            while B >= N_SUB:
                cp, hp = fm_level(v2(cp), hp, B, lvl)
                B >>= 1
                lvl += 1

            # cp = root c [128, 2*N_SUB] f32, hp = root h (unsplit) f32
            # p-major out; c's DMA overlaps the final h chain
            nc.sync.dma_start(out[:, 0:2 * N_SUB], cp[:, :])
            nc.sync.dma_start(out[:, 2 * N_SUB:4 * N_SUB], hp[:, :])

    nc.compile()
    return nc


def _get_nc():
    if "nc" not in _CACHE:
        _CACHE["nc"] = _build()
    return _CACHE["nc"]


def kernel(embs, Wx, bx, Wl, Wr, emb_table, _trace=False, _trace_kwargs=None):
    from concourse.bass_utils import run_bass_kernel_spmd

    fp16 = np.float16
    embs = np.asarray(embs, dtype=np.float32)
    Wx = np.asarray(Wx, dtype=np.float32)
    bx = np.asarray(bx, dtype=np.float32)
    Wl = np.asarray(Wl, dtype=np.float32)
    Wr = np.asarray(Wr, dtype=np.float32)
    emb_table = np.asarray(emb_table, dtype=np.float32)

    # u(cx), i(ix), o(ox) gate rows of Wx, transposed, K-padded to 384;
    # row 300 carries bx (leaf inputs have a matching 1.0 in row 300)
    Wxuio = np.concatenate([Wx[0:256], Wx[256:512], Wx[768:1024]], axis=0)
    bxuio = np.concatenate([bx[0:512], bx[768:1024]])
    WxT = np.zeros((KPAD, 768), dtype=fp16)
    WxT[:IN_DIM] = Wxuio.T.astype(fp16)
    WxT[IN_DIM] = bxuio.astype(fp16)
    WlT = np.ascontiguousarray(Wl.T.astype(fp16))
    WrT = np.ascontiguousarray(Wr.T.astype(fp16))

    # merge-gate bias columns: px m-chunks permuted to (gate, half) order
    px = emb_table[-1] @ Wx.T + bx                         # [1024]
    pxm = px.reshape(8, 128)
    pxf = np.ascontiguousarray(pxm[_PXCOL].T)              # [128, 10] f32
    pxrow = np.ascontiguousarray(
        pxm[_PXCOL].reshape(1, 1280).astype(fp16))         # [1, 1280]

    in_maps = []
    for d in range(N_CORES):
        shard = np.zeros((KPAD, LPC), dtype=fp16)
        shard[:IN_DIM] = embs[d * LPC:(d + 1) * LPC].T.astype(fp16)
        shard[IN_DIM] = 1.0
        in_maps.append({
            "embsT": shard, "WxT": WxT, "WlT": WlT, "WrT": WrT,
            "pxf": pxf, "pxrow": pxrow,
        })

    nc = _get_nc()
    res = run_bass_kernel_spmd(nc, in_maps, list(range(N_CORES)),
                               trace=_trace, **(_trace_kwargs or {}))
    _CACHE["last_result"] = res

    # unshard: 128 subtree roots -> 7 numpy merge levels (127 nodes)
    cs, hs = [], []
    for d in range(N_CORES):
        o = np.asarray(res.results[d]["out"], dtype=np.float32)
        # o[p, half*N_SUB + n] -> feat = half*128 + p
        cs.append(o[:, 0:2 * N_SUB].reshape(128, 2, N_SUB)
                  .transpose(2, 1, 0).reshape(N_SUB, 256))
        hs.append(o[:, 2 * N_SUB:4 * N_SUB].reshape(128, 2, N_SUB)
                  .transpose(2, 1, 0).reshape(N_SUB, 256))
    c = np.concatenate(cs, axis=0)  # [128, 256]
    h = np.concatenate(hs, axis=0)
    WlTf = Wl.T.astype(np.float32)
    WrTf = Wr.T.astype(np.float32)
    m = MEM_DIM

    def sig(x):
        return 1.0 / (1.0 + np.exp(-x))

    while c.shape[0] > 1:
        lg = h[0::2] @ WlTf
        rg = h[1::2] @ WrTf
        u = np.tanh(px[0:m] + lg[:, 0:m] + rg[:, 0:m])
        i = sig(px[m:2 * m] + lg[:, m:2 * m] + rg[:, m:2 * m])
        lf = sig(px[2 * m:3 * m] + lg[:, 2 * m:3 * m] + rg[:, 2 * m:3 * m])
        rf = sig(px[2 * m:3 * m] + lg[:, 3 * m:4 * m] + rg[:, 3 * m:4 * m])
        o = sig(px[3 * m:4 * m] + lg[:, 4 * m:5 * m] + rg[:, 4 * m:5 * m])
        c = i * u + lf * c[0::2] + rf * c[1::2]
        h = o * np.tanh(c)
    return np.stack([c, h]).astype(np.float32)
